# revision 5
# baseline (speedup 1.0000x reference)
"""Trainium2 Bass kernel for nn_DistillationLoss.

Computes KLDivLoss(batchmean) between a temperature-softened student
log-softmax and a sparse scattered teacher target, as in the reference:

    loss = (T^2/B) * sum_b [ sum_j t*log t - sum_j t*s/T + logsumexp(s_b/T) ]

with t the row-normalized scatter of teacher_scores into local columns
(plus a diagonal 1.0), using sum_j t_bj = 1.

Device work (8 NeuronCores, data-parallel over rows):
  - stream the 1024x8192 f32 row-shard through SBUF; per row compute
    max (DVE) and sum of exp((s-m)/T) via ScalarE activation accumulate
  - gather the sparse target entries' s values straight from the HBM
    shard with one SWDGE indirect (element-granularity) DMA, then
    dot with the target values t and reduce (DVE)
  - t*log(t) entropy term via ScalarE Ln + DVE
Host work is index/metadata preparation only (global->local remap,
scatter dedup, row sums, packing the gather list) plus the final O(B)
reduction of per-partition partials.
"""

import os

import numpy as np

TEMP = 2.0
N_GLOBAL = 16384
N_CORES = 8
P = 128

# Fixed gather capacity per partition: worst case (all teacher entries
# valid) is rows_per_core*(K+1) = 1024*51 = 52224 = 128*408 slots.
_G_SLOTS = 408

LAST_RESULT = None  # BassKernelResults of the most recent run (for test.py)

_NC_CACHE: dict = {}


def _build_nc(rows: int, cols: int, g_slots: int):
    from concourse import bacc, bass, mybir
    import concourse.tile as tile

    f32 = mybir.dt.float32
    i32 = mybir.dt.int32
    AF = mybir.ActivationFunctionType
    AX = mybir.AxisListType

    n_tiles = rows // P
    assert rows % P == 0

    nc = bacc.Bacc(trn_type="TRN2")
    n_flat = rows * cols
    s = nc.dram_tensor("s_shard", [n_flat], f32, kind="ExternalInput")
    gidx = nc.dram_tensor("gath_idx", [P, g_slots], i32, kind="ExternalInput")
    gt = nc.dram_tensor("gath_t", [P, g_slots], f32, kind="ExternalInput")
    ncols_out = 4 + 2 * n_tiles
    out = nc.dram_tensor("partials", [P, ncols_out], f32, kind="ExternalOutput")

    s_rows = s[:].rearrange("(r c) -> r c", c=cols)

    with tile.TileContext(nc) as tc:
        with (
            tc.tile_pool(name="big", bufs=3) as bigp,
            tc.tile_pool(name="expool", bufs=1) as exp_pool,
            tc.tile_pool(name="small", bufs=1) as smp,
            tc.tile_pool(name="loop_small", bufs=4) as lsp,
        ):
            # ---- sparse target path: gather s at target positions ----
            idx_t = smp.tile([P, g_slots], i32)
            nc.sync.dma_start(out=idx_t[:], in_=gidx[:, :])
            t_t = smp.tile([P, g_slots], f32)
            nc.sync.dma_start(out=t_t[:], in_=gt[:, :])
            gath = smp.tile([P, g_slots], f32)
            nc.gpsimd.indirect_dma_start(
                out=gath[:],
                out_offset=None,
                in_=s[:, None],
                in_offset=bass.IndirectOffsetOnAxis(ap=idx_t[:], axis=0),
            )
            prod = smp.tile([P, g_slots], f32)
            nc.vector.tensor_mul(out=prod[:], in0=gath[:], in1=t_t[:])
            sdot = smp.tile([P, 1], f32)
            nc.vector.reduce_sum(out=sdot[:], in_=prod[:], axis=AX.X)

            # entropy term: t*ln(t), with padded t=0 contributing exactly 0
            tmax = smp.tile([P, g_slots], f32)
            nc.vector.tensor_scalar_max(out=tmax[:], in0=t_t[:], scalar1=1e-30)
            tln = smp.tile([P, g_slots], f32)
            nc.scalar.activation(out=tln[:], in_=tmax[:], func=AF.Ln)
            tlt = smp.tile([P, g_slots], f32)
            nc.vector.tensor_mul(out=tlt[:], in0=t_t[:], in1=tln[:])
            hsum = smp.tile([P, 1], f32)
            nc.vector.reduce_sum(out=hsum[:], in_=tlt[:], axis=AX.X)

            # ---- streaming logsumexp over the row shard ----
            E_all = smp.tile([P, n_tiles], f32)
            M_all = smp.tile([P, n_tiles], f32)
            for i in range(n_tiles):
                st = bigp.tile([P, cols], f32, tag="st")
                nc.sync.dma_start(out=st[:], in_=s_rows[i * P : (i + 1) * P, :])
                nc.vector.reduce_max(out=M_all[:, i : i + 1], in_=st[:], axis=AX.X)
                negb = lsp.tile([P, 1], f32, tag="negb")
                nc.vector.tensor_scalar_mul(
                    out=negb[:], in0=M_all[:, i : i + 1], scalar1=-1.0 / TEMP
                )
                ex = exp_pool.tile([P, cols], f32, tag="ex")
                nc.scalar.activation(
                    out=ex[:],
                    in_=st[:],
                    func=AF.Exp,
                    bias=negb[:],
                    scale=1.0 / TEMP,
                    accum_out=E_all[:, i : i + 1],
                )

            lnE = smp.tile([P, n_tiles], f32)
            nc.scalar.activation(out=lnE[:], in_=E_all[:], func=AF.Ln)
            lse_t = smp.tile([P, n_tiles], f32)
            nc.vector.tensor_scalar_mul(out=lse_t[:], in0=M_all[:], scalar1=1.0 / TEMP)
            nc.vector.tensor_add(out=lse_t[:], in0=lse_t[:], in1=lnE[:])
            lse_sum = smp.tile([P, 1], f32)
            nc.vector.reduce_sum(out=lse_sum[:], in_=lse_t[:], axis=AX.X)

            # ---- assemble per-partition partials ----
            ob = smp.tile([P, ncols_out], f32)
            nc.vector.memset(ob[:], 0.0)
            nc.vector.tensor_copy(out=ob[:, 0:1], in_=sdot[:])
            nc.vector.tensor_copy(out=ob[:, 1:2], in_=hsum[:])
            nc.vector.tensor_copy(out=ob[:, 2:3], in_=lse_sum[:])
            nc.vector.tensor_copy(out=ob[:, 4 : 4 + n_tiles], in_=E_all[:])
            nc.vector.tensor_copy(
                out=ob[:, 4 + n_tiles : 4 + 2 * n_tiles], in_=M_all[:]
            )
            nc.sync.dma_start(out=out[:, :], in_=ob[:])

    nc.compile()
    return nc


def _get_nc(rows: int, cols: int, g_slots: int):
    key = (rows, cols, g_slots)
    if key not in _NC_CACHE:
        _NC_CACHE[key] = _build_nc(rows, cols, g_slots)
    return _NC_CACHE[key]


def _host_prep(batch_indices, teacher_indices, teacher_scores, B, cols):
    """Build the per-core gather lists (flat index into the row shard, and
    the target probability t for that position). Metadata only — never
    touches student_logits."""
    bi = np.asarray(batch_indices).astype(np.int64).ravel()
    ti = np.asarray(teacher_indices).astype(np.int64)
    ts = np.asarray(teacher_scores).astype(np.float64)
    K = ti.shape[1]

    g2l = np.full(N_GLOBAL, -1, np.int64)
    g2l[np.clip(bi, 0, N_GLOBAL - 1)] = np.arange(B)

    inb = (ti >= 0) & (ti < N_GLOBAL)
    loc = np.where(inb, g2l[np.clip(ti, 0, N_GLOBAL - 1)], -1)  # [B, K]
    valid = (loc >= 0).ravel()

    rows_e = np.repeat(np.arange(B), K)[valid]
    cols_e = loc.ravel()[valid]
    ks_e = np.tile(np.arange(K), B)[valid]
    w_e = ts.ravel()[valid]

    # scatter .set semantics: for duplicate (row, col), last k wins
    order = np.lexsort((ks_e, cols_e, rows_e))
    rows_e, cols_e, w_e = rows_e[order], cols_e[order], w_e[order]
    keys = rows_e * cols + cols_e
    last = np.ones(len(keys), bool)
    if len(keys) > 1:
        last[:-1] = keys[1:] != keys[:-1]
    rows_e, cols_e, w_e = rows_e[last], cols_e[last], w_e[last]

    # the diagonal is overwritten with 1.0 after the scatter
    nd = cols_e != rows_e
    rows_e, cols_e, w_e = rows_e[nd], cols_e[nd], w_e[nd]

    # row sums R_b = 1.0 (diag) + sum of surviving scattered scores
    R = np.ones(B, np.float64)
    np.add.at(R, rows_e, w_e)

    t_e = w_e / R[rows_e]

    rpc = B // N_CORES
    gidx_list, gt_list = [], []
    for m in range(N_CORES):
        sel = (rows_e >= m * rpc) & (rows_e < (m + 1) * rpc)
        rr = rows_e[sel] - m * rpc
        cc = cols_e[sel]
        tt = t_e[sel]
        di = np.arange(rpc)
        gr = np.concatenate([rr, di])
        gc = np.concatenate([cc, m * rpc + di])
        gv = np.concatenate([tt, 1.0 / R[m * rpc : (m + 1) * rpc]])
        flat = gr * cols + gc
        n = len(flat)
        assert n <= P * _G_SLOTS
        idx_arr = np.zeros(P * _G_SLOTS, np.int32)
        t_arr = np.zeros(P * _G_SLOTS, np.float32)
        idx_arr[:n] = flat
        t_arr[:n] = gv
        gidx_list.append(idx_arr.reshape(P, _G_SLOTS))
        gt_list.append(t_arr.reshape(P, _G_SLOTS))
    return gidx_list, gt_list


def kernel(**inputs) -> np.ndarray:
    global LAST_RESULT
    from concourse.bass_utils import run_bass_kernel_spmd

    student_logits = np.asarray(inputs["student_logits"])
    if student_logits.dtype != np.float32:
        student_logits = student_logits.astype(np.float32)
    B, cols = student_logits.shape
    assert B % (N_CORES * P) == 0
    rpc = B // N_CORES

    gidx_list, gt_list = _host_prep(
        inputs["batch_indices"],
        inputs["teacher_indices"],
        inputs["teacher_scores"],
        B,
        cols,
    )

    nc = _get_nc(rpc, cols, _G_SLOTS)

    sl = np.ascontiguousarray(student_logits)
    in_maps = []
    for m in range(N_CORES):
        in_maps.append(
            {
                "s_shard": sl[m * rpc : (m + 1) * rpc, :].reshape(-1),
                "gath_idx": gidx_list[m],
                "gath_t": gt_list[m],
            }
        )

    trace = bool(os.environ.get("BASS_KERNEL_TRACE"))
    if trace:
        try:
            import antenv.axon_hooks  # noqa: F401
        except ImportError:
            trace = False
    res = run_bass_kernel_spmd(
        nc, in_maps, core_ids=list(range(N_CORES)), trace=trace
    )
    LAST_RESULT = res

    partials = np.stack([r["partials"] for r in res.results]).astype(np.float64)
    S = partials[:, :, 0].sum()
    H = partials[:, :, 1].sum()
    LSE = partials[:, :, 2].sum()
    loss = (TEMP * TEMP / B) * (H - S / TEMP + LSE)
    return np.float32(loss)
